# revision 5
# baseline (speedup 1.0000x reference)
"""Multi-head attention (B=2, S=4096, D=768, H=12) on 8 Trainium2 cores.

Sharding: core c -> batch b = c // 4, head-triple g = c % 4 (heads 3g..3g+2).
Each core computes its QKV projections (columns of W for its heads) and
flash-style attention for its 3 heads, fully on-chip; no cross-core comms.
Host-side prep per core: slice batch/head-group, cast x/W to fp16 (the device
kernel computes in fp16 with f32 accumulation; fp16 over bf16 because all
value ranges here are small, giving ~8x lower quantization error at identical
PE throughput; casting host-side also halves the transfer volume).

Per-core device kernel:
  - x^T tiles [128(d), 6(dchunk), 1024(s)] via xbar DMA-transpose straight
    from the fp16 DRAM inputs, quarter by quarter; projections chase each
    quarter so ScalarE attention work starts ~tens of us in.
  - projections on PE produce qT/kT [64, 3, 4096] and v_ext
    [128, 32, 3, 65] (col 64 = ones so the PV matmul accumulates the softmax
    denominator as output row 64). The attention mask enters as a per-k scale
    em = exp(-1e4*(1-mask)) folded into v_ext (exact: softmax with additive
    -1e4 adder == scaling exp(s) by em, including the denominator).
  - attention per (head, 512-wide q chunk): 32 k-chunks in groups of 3;
    QK^T -> PSUM, exp on ScalarE (scale=1/8) -> fp16 probs in SBUF,
    PV accumulate -> PSUM [65, 512]; then PE-transpose to natural layout and
    DVE normalize by the reciprocal of the denominator column (row 64).
"""

import sys

if "/opt/trn_rl_repo" not in sys.path:
    sys.path.insert(0, "/opt/trn_rl_repo")

from contextlib import ExitStack

import ml_dtypes
import numpy as np

import concourse.bass as bass
import concourse.tile as tile
from concourse import bacc, mybir
from concourse.bass_utils import run_bass_kernel_spmd
from concourse.masks import make_identity

F32 = mybir.dt.float32
# fp16 instead of bf16: all on-chip value ranges here are tiny (|x|<6,
# |W|<0.12, probs<8), so fp16's 10 mantissa bits cut quantization error ~4x
# at identical PE throughput (1 cycle/row) and xbar 2-byte transpose support
BF16 = mybir.dt.float16
AF = mybir.ActivationFunctionType
ALU = mybir.AluOpType
BF16_NP = np.float16

B, S, D, H, DK = 2, 4096, 768, 12, 64
N_CORES = 8
HPG = 3            # heads per core
GD = HPG * DK      # 192 output columns per core
SQ = 512           # q-chunk width
NSQ = S // SQ      # 8
KCW = 128          # k-chunk width
NKC = S // KCW     # 32
GRP = 3            # k-chunks per exp group (3 PSUM banks, double buffered)
NDC = D // 128     # 6 contraction chunks
QTR = S // 4       # transpose/projection pipeline granularity
SQQ = NSQ // 4     # q chunks per quarter
SCQ = NKC // 4     # s chunks per quarter


def _emit(ctx: ExitStack, tc: tile.TileContext, io: dict):
    nc = tc.nc

    const = ctx.enter_context(tc.tile_pool(name="const", bufs=1))
    xt_pool = ctx.enter_context(tc.tile_pool(name="xt", bufs=5))
    proj = ctx.enter_context(tc.tile_pool(name="proj", bufs=1))
    scores_pool = ctx.enter_context(tc.tile_pool(name="scores", bufs=2, space="PSUM"))
    aux_psum = ctx.enter_context(tc.tile_pool(name="auxp", bufs=2, space="PSUM"))
    probs_pool = ctx.enter_context(tc.tile_pool(name="probs", bufs=5))
    outt_pool = ctx.enter_context(tc.tile_pool(name="outt", bufs=2))
    small = ctx.enter_context(tc.tile_pool(name="small", bufs=2))
    oslab_pool = ctx.enter_context(tc.tile_pool(name="oslab", bufs=3))
    # DVE-exp offload scratch: x tile, Horner ping-pong, and probs output
    dx_pool = ctx.enter_context(tc.tile_pool(name="dx", bufs=2))
    dh_pool = ctx.enter_context(tc.tile_pool(name="dh", bufs=3))
    prd_pool = ctx.enter_context(tc.tile_pool(name="prd", bufs=2))

    # ---- constants / small inputs (consolidated to limit 4KB slot padding) ----
    # mask -> per-k scale em = exp(-1e4 * (1 - mask)), [128, 32] (p, kchunk).
    # Emitted FIRST so the ACT exp-table load lands at the head of the queues.
    mask_em = const.tile([128, 65], F32, name="mask_em")
    mask_t = mask_em[:, 0:32]
    em_sb = mask_em[:, 32:64]
    neg1e4 = mask_em[:, 64:65]
    nc.gpsimd.memset(neg1e4, -10000.0)
    nc.scalar.dma_start(mask_t, io["mask_pk"][:])
    nc.scalar.activation(em_sb, mask_t, AF.Exp, scale=10000.0, bias=neg1e4)

    # weights loaded contiguously (q | k | v along free dim)
    w_all = const.tile([128, NDC, 3 * GD], BF16, name="w_all")
    for i, nm in ((1, "wk"), (0, "wq"), (2, "wv")):
        nc.scalar.dma_start(
            w_all[:, :, i * GD : (i + 1) * GD],
            io[nm].rearrange("(dc p) n -> p dc n", p=128),
        )
    wv_sb = w_all[:, :, 2 * GD : 3 * GD]

    # q/k weights with each head's 64 columns duplicated (projection then
    # replicates qT/kT on both partition halves at no extra PE cost)
    w_dup = const.tile([128, NDC, 2, HPG, 128], BF16, name="w_dup")
    for i in (1, 0):
        for h in range(HPG):
            for rep in range(2):
                nc.vector.tensor_copy(
                    w_dup[:, :, i, h, rep * DK : (rep + 1) * DK],
                    w_all[:, :, i * GD + h * DK : i * GD + (h + 1) * DK],
                )

    bqbk = const.tile([128, 2 * HPG], F32, name="bqbk")
    nc.scalar.dma_start(bqbk[:], io["bqbk_pk"][:])

    bfpack = const.tile([1, 320], BF16, name="bfpack")
    nc.gpsimd.memset(bfpack[:, 0:128], 1.0)
    nc.scalar.dma_start(bfpack[:, 128 : 128 + GD], io["bv_r"][:])
    ones_row = bfpack[:, 0:128]
    bv_sb = bfpack[:, 128 : 128 + GD]

    ident = const.tile([128, 128], F32, name="ident")
    make_identity(nc, ident[:])

    # ---- persistent projection outputs (qT/kT replicated on both halves) ----
    qT = proj.tile([128, HPG, S], BF16, name="qT")
    kT = proj.tile([128, HPG, S], BF16, name="kT")
    vE = proj.tile([128, NKC, HPG, DK + 1], BF16, name="vE")
    nc.gpsimd.memset(vE[:], 1.0)  # ones col 64; data cols overwritten below

    # ---- per-quarter: transpose + project ----
    def load_xt_quarter(nm, qq):
        # host supplies x d-chunk-major [6*4096, 128] so each xbar transpose
        # reads a fully contiguous [1024, 128] block
        xt = xt_pool.tile([128, NDC, QTR], BF16, tag="xt", name=f"xt_{nm}_{qq}")
        for dc in range(NDC):
            base = dc * S + qq * QTR
            nc.sync.dma_start(
                out=xt[:, dc, :], in_=io[nm][base : base + QTR, :],
                transpose=True,
            )
        return xt

    def proj_qk(xt, qq, wi, bias, dst):
        for h in range(HPG):
            for sqq in range(SQQ):
                sq = qq * SQQ + sqq
                ps = aux_psum.tile([128, SQ], F32, tag="aux", name=f"ps_{qq}_{h}_{sqq}")
                for dc in range(NDC):
                    nc.tensor.matmul(
                        ps[:],
                        lhsT=w_dup[:, dc, wi, h, :],
                        rhs=xt[:, dc, sqq * SQ : (sqq + 1) * SQ],
                        start=(dc == 0),
                        stop=(dc == NDC - 1),
                    )
                nc.vector.tensor_scalar(
                    dst[:, h, sq * SQ : (sq + 1) * SQ], ps[:],
                    bias[:, h : h + 1], None, ALU.add,
                )

    def proj_v(xt, qq):
        for scq in range(SCQ):
            sc = qq * SCQ + scq
            ps = aux_psum.tile([128, GD], F32, tag="aux", name=f"psv_{qq}_{scq}")
            for dc in range(NDC):
                nc.tensor.matmul(
                    ps[:],
                    lhsT=xt[:, dc, scq * 128 : (scq + 1) * 128],
                    rhs=wv_sb[:, dc, :],
                    start=(dc == 0),
                    stop=False,
                )
            nc.tensor.matmul(
                ps[:], lhsT=ones_row[:, 0:128], rhs=bv_sb[:], start=False, stop=True
            )
            for h in range(HPG):
                nc.vector.tensor_copy(
                    vE[:, sc, h, 0:DK], ps[:, h * DK : (h + 1) * DK]
                )
            # fold mask scale into v and the denominator ones column
            nc.vector.tensor_scalar(
                vE[:, sc, :, :], vE[:, sc, :, :], em_sb[:, sc : sc + 1], None,
                ALU.mult,
            )

    def proj_kv_quarter(qq):
        xt_k = load_xt_quarter("xk", qq)
        proj_qk(xt_k, qq, 1, bqbk[:, HPG : 2 * HPG], kT)
        xt_v = load_xt_quarter("xv", qq)
        proj_v(xt_v, qq)

    def proj_q_group(xt, qq, h, sqq):
        # borrows a scores-pool slot: the aux pool's two slots hold live pv
        # accumulators / tr tiles during attention (a third tenant would
        # deadlock the in-order PE queue on slot reuse)
        sq = qq * SQQ + sqq
        ps = scores_pool.tile([128, SQ], F32, tag="scores", name=f"psq_{qq}_{h}_{sqq}")
        for dc in range(NDC):
            nc.tensor.matmul(
                ps[:],
                lhsT=w_dup[:, dc, 0, h, :],
                rhs=xt[:, dc, sqq * SQ : (sqq + 1) * SQ],
                start=(dc == 0),
                stop=(dc == NDC - 1),
            )
        nc.vector.tensor_scalar(
            qT[:, h, sq * SQ : (sq + 1) * SQ], ps[:],
            bqbk[:, h : h + 1], None, ALU.add,
        )

    # ---- attention ----
    groups = []
    g0 = 0
    while g0 < NKC:
        groups.append((g0, min(GRP, NKC - g0)))
        g0 += GRP

    # ScalarE exp is the kernel's critical engine (~375us of the wall); shift
    # the last 3 k-chunks of each iteration (except the first) to a DVE
    # degree-6 Horner polynomial for exp(s/8) on [-2.3, 2.3] (max rel err
    # ~9e-4 exact, ~2.6e-3 with fp16 intermediates, on 3/32 of the keys).
    DVE_KC = (29, 30, 31)
    C_EXP = (1.000824060890309, 0.9996469799097945, 0.49552294276165537,
             0.16473947810651934, 0.04418772554978006, 0.009769214318998872,
             0.001175456119703543)

    def emit_dve_qk_x(h, sq, it):
        # QK for the offloaded chunks + PSUM->fp16 x = s/8 (frees the PSUM
        # bank after one DVE pass). Emitted at the END of the previous
        # iteration so the DVE drains the bank before ACT groups need it.
        sc_d = scores_pool.tile(
            [128, GRP * SQ], F32, tag="scores", name=f"scd_{it}"
        )
        for j, kc in enumerate(DVE_KC):
            ho = 64 * (kc % 2)
            nc.tensor.matmul(
                sc_d[:, j * SQ : (j + 1) * SQ],
                lhsT=kT[ho : ho + DK, h, kc * KCW : (kc + 1) * KCW],
                rhs=qT[ho : ho + DK, h, sq * SQ : (sq + 1) * SQ],
                start=True,
                stop=True,
            )
        xD = dx_pool.tile([128, GRP * SQ], BF16, tag="dx", name=f"dx_{it}")
        nc.vector.tensor_scalar(xD[:], sc_d[:], 0.125, None, ALU.mult)
        prD = prd_pool.tile([128, GRP * SQ], BF16, tag="prd", name=f"prd_{it}")
        return xD, prD

    def emit_dve_poly(xD, prD, it):
        # Horner: h = c6*x + c5; h = h*x + c4; ...; pr = h*x + c0
        h0 = dh_pool.tile([128, GRP * SQ], BF16, tag="dh", name=f"dh0_{it}")
        nc.vector.tensor_scalar(h0[:], xD[:], C_EXP[6], C_EXP[5], ALU.mult, ALU.add)
        cur = h0
        for step, ck in enumerate((C_EXP[4], C_EXP[3], C_EXP[2], C_EXP[1])):
            m = dh_pool.tile([128, GRP * SQ], BF16, tag="dh", name=f"dhm{step}_{it}")
            nc.vector.tensor_tensor(m[:], cur[:], xD[:], ALU.mult)
            a = dh_pool.tile([128, GRP * SQ], BF16, tag="dh", name=f"dha{step}_{it}")
            nc.vector.tensor_scalar(a[:], m[:], ck, None, ALU.add)
            cur = a
        m = dh_pool.tile([128, GRP * SQ], BF16, tag="dh", name=f"dhm4_{it}")
        nc.vector.tensor_tensor(m[:], cur[:], xD[:], ALU.mult)
        nc.vector.tensor_scalar(prD[:], m[:], C_EXP[0], None, ALU.add)

    pending = None  # finalize closure for the previous (h, sq)

    def make_finalize(pv, h, sq):
        def fin():
            ot = outt_pool.tile([DK + 1, SQ], F32, tag="outt", name=f"ot_{h}_{sq}")
            nc.vector.tensor_copy(ot[:], pv[:])
            tr = aux_psum.tile([128, 4 * (DK + 1)], F32, tag="aux", name=f"tr_{h}_{sq}")
            for t in range(4):
                nc.tensor.transpose(
                    tr[:, t * (DK + 1) : (t + 1) * (DK + 1)],
                    ot[:, t * 128 : (t + 1) * 128],
                    ident[0 : DK + 1, 0 : DK + 1],
                )
            rc = small.tile([128, 4], F32, tag="recip", name=f"rc_{h}_{sq}")
            osl = oslab_pool.tile([128, 4, DK], F32, tag="oslab", name=f"os_{h}_{sq}")
            for t in range(4):
                nc.vector.reciprocal(
                    rc[:, t : t + 1], tr[:, t * (DK + 1) + DK : t * (DK + 1) + DK + 1]
                )
                nc.vector.tensor_scalar(
                    osl[:, t, :],
                    tr[:, t * (DK + 1) : t * (DK + 1) + DK],
                    rc[:, t : t + 1],
                    None,
                    ALU.mult,
                )
            nc.gpsimd.dma_start(
                out=io["out"].rearrange(
                    "(sq t p) n -> sq p t n", sq=NSQ, t=4, p=128
                )[sq, :, :, h * DK : (h + 1) * DK],
                in_=osl[:],
            )
        return fin

    # Boundary tasks: q-quarter transposes + projection groups for sq>=2 are
    # deferred into the attention phase (PE has per-group slack there), keyed
    # by the flat iteration index after which they are emitted.
    boundary_tasks = {}
    xt_q_tiles = {}

    def sched(it, fn):
        boundary_tasks.setdefault(it, []).append(fn)

    def tr_q(qq):
        def fn():
            xt_q_tiles[qq] = load_xt_quarter("xq", qq)
        return fn

    def pg(qq, h, sqq):
        def fn():
            proj_q_group(xt_q_tiles[qq], qq, h, sqq)
        return fn

    # task scheduled at boundary b fires during iteration b+1 (after its first
    # exp group), so pg for (h, sq) must sit at boundary <= idx(h, sq) - 2
    DEFER_PG = __import__("os").environ.get("BASS_DEFER_PG", "0") == "1"
    if DEFER_PG:
        sched(0, pg(1, 0, 0))
        sched(1, pg(1, 0, 1))
        sched(2, pg(2, 0, 0))
        sched(3, pg(2, 0, 1))
        sched(4, pg(3, 0, 0))
        sched(5, pg(3, 0, 1))
        nb = 6
        for h in (1, 2):
            for qq in (1, 2, 3):
                for sqq in range(SQQ):
                    sched(nb, pg(qq, h, sqq))
                    nb += 1

    def attention_gen():
        nonlocal_pending = [None]

        def emit_pv(pv, h, grp):
            p0, plen, ppr = grp
            for j in range(plen):
                kc = p0 + j
                nc.tensor.matmul(
                    pv[:],
                    lhsT=vE[:, kc, h, :],
                    rhs=ppr[:, j * SQ : (j + 1) * SQ],
                    start=(kc == 0),
                    stop=(kc == NKC - 1),
                )

        carry = None  # (pv, h, [groups]) tail-PV work carried across iterations
        it = 0
        iters = [(h, sq) for h in range(HPG) for sq in range(NSQ)]
        pending_dve = None  # (xD, prD) prepared for the iteration about to run
        for h, sq in iters:
            dve_state, pending_dve = pending_dve, None
            if dve_state is not None:
                act_groups = [(g, GRP) for g in range(0, 27, GRP)] + [(27, 2)]
            else:
                act_groups = groups
            pv = aux_psum.tile([DK + 1, SQ], F32, tag="aux", name=f"pv_{h}_{sq}")
            ready = []  # (kc0, glen, probs) groups awaiting PV emission
            for gi, (kc0, glen) in enumerate(act_groups):
                sc = scores_pool.tile(
                    [128, glen * SQ], F32, tag="scores",
                    name=f"sc_{h}_{sq}_{gi}",
                )
                for j in range(glen):
                    kc = kc0 + j
                    # alternate PE row groups by kc parity: qT/kT are
                    # replicated on partitions 64-127, so odd chunks read
                    # the upper half and land on row tiles (64,0) — the PE
                    # runs the two K=64 matmuls concurrently (row tiling)
                    ho = 64 * (kc % 2)
                    nc.tensor.matmul(
                        sc[:, j * SQ : (j + 1) * SQ],
                        lhsT=kT[ho : ho + DK, h, kc * KCW : (kc + 1) * KCW],
                        rhs=qT[ho : ho + DK, h, sq * SQ : (sq + 1) * SQ],
                        start=True,
                        stop=True,
                    )
                pr = probs_pool.tile(
                    [128, glen * SQ], BF16, tag="probs",
                    name=f"pr_{h}_{sq}_{gi}",
                )
                nc.scalar.activation(pr[:], sc[:], AF.Exp, scale=0.125)
                ready.append((kc0, glen, pr))
                if gi == 0 and carry is not None:
                    cpv, ch, cgrps = carry
                    for grp in cgrps:
                        emit_pv(cpv, ch, grp)
                    carry = None
                    for fn in boundary_tasks.get(it - 1, ()):
                        fn()
                if gi == 1:
                    if nonlocal_pending[0] is not None:
                        nonlocal_pending[0]()
                        nonlocal_pending[0] = None
                    if dve_state is not None:
                        emit_dve_poly(dve_state[0], dve_state[1], it)
                if len(ready) >= 2:
                    emit_pv(pv, h, ready.pop(0))
                yield (h, sq, gi)
            if it + 1 < len(iters):
                nh, nsq = iters[it + 1]
                pending_dve = emit_dve_qk_x(nh, nsq, it + 1)
            carry_grps = list(ready)
            if dve_state is not None:
                carry_grps.append((DVE_KC[0], len(DVE_KC), dve_state[1]))
            carry = (pv, h, carry_grps)
            nonlocal_pending[0] = make_finalize(pv, h, sq)
            it += 1

        cpv, ch, cgrps = carry
        for grp in cgrps:
            emit_pv(cpv, ch, grp)
        nonlocal_pending[0]()

    # Interleave k/v projection quarters with the first attention iteration's
    # k-chunk groups so ScalarE saturates early: group gi covers kc
    # 3*gi..3*gi+2, requiring k/v quarters up to (3*gi+2)//8; the first
    # iteration uses q chunk sq=0 (quarter 0).
    gen = attention_gen()

    def advance(n):
        for _ in range(n):
            if next(gen, None) is None:
                break

    # quarter 0 in k, q, v order: the first QK group needs kT+qT only (the
    # first PV trails by two exp groups, so v can land a little later)
    xt_k0 = load_xt_quarter("xk", 0)
    proj_qk(xt_k0, 0, 1, bqbk[:, HPG : 2 * HPG], kT)
    xt_q_tiles[0] = load_xt_quarter("xq", 0)
    for h in range(HPG):
        for sqq in range(SQQ):
            proj_q_group(xt_q_tiles[0], 0, h, sqq)
    advance(1)       # g0: kc 0..2 (needs only kT+qT of quarter 0)
    xt_v0 = load_xt_quarter("xv", 0)
    proj_v(xt_v0, 0)
    advance(1)       # g1: kc 3..5 (first PV fires after this exp)
    proj_kv_quarter(1)
    xt_q_tiles[1] = load_xt_quarter("xq", 1)
    if not DEFER_PG:
        for h in range(HPG):
            for sqq in range(SQQ):
                proj_q_group(xt_q_tiles[1], 1, h, sqq)
    advance(3)       # g2..g4: kc 6..14 (quarters 0-1)
    proj_kv_quarter(2)
    xt_q_tiles[2] = load_xt_quarter("xq", 2)
    if not DEFER_PG:
        for h in range(HPG):
            for sqq in range(SQQ):
                proj_q_group(xt_q_tiles[2], 2, h, sqq)
    advance(3)       # g5..g7: kc 15..23 (quarter 2)
    proj_kv_quarter(3)
    xt_q_tiles[3] = load_xt_quarter("xq", 3)
    if not DEFER_PG:
        for h in range(HPG):
            for sqq in range(SQQ):
                proj_q_group(xt_q_tiles[3], 3, h, sqq)
    for _ in gen:
        pass


def _build():
    nc = bacc.Bacc("TRN2", target_bir_lowering=False, debug=False)
    io = {}
    for nm, shape, dt in (
        ("xq", [NDC * S, 128], BF16), ("xk", [NDC * S, 128], BF16),
        ("xv", [NDC * S, 128], BF16),
        ("wq", [D, GD], BF16), ("wk", [D, GD], BF16), ("wv", [D, GD], BF16),
        ("bqbk_pk", [128, 2 * HPG], F32),
        ("bv_r", [1, GD], BF16), ("mask_pk", [128, NKC], F32),
    ):
        io[nm] = nc.dram_tensor(nm, shape, dt, kind="ExternalInput").ap()
    io["out"] = nc.dram_tensor("out", [S, GD], F32, kind="ExternalOutput").ap()

    import os

    dup = int(os.environ.get("BASS_DUP", "1"))
    with tile.TileContext(nc) as tc:
        for _ in range(dup):
            with ExitStack() as ctx:
                _emit(ctx, tc, io)
    nc.compile()
    return nc


_NC = None


def _get_nc():
    global _NC
    if _NC is None:
        _NC = _build()
    return _NC


def make_in_maps(query, key, value, mask, Wq, bq, Wk, bk, Wv, bv):
    bf = lambda a: np.ascontiguousarray(a).astype(BF16_NP)
    bf3 = lambda a: np.ascontiguousarray(
        np.asarray(a).reshape(S, NDC, 128).transpose(1, 0, 2).reshape(NDC * S, 128)
    ).astype(BF16_NP)
    f32 = lambda a: np.ascontiguousarray(np.asarray(a, np.float32))
    in_maps = []
    for c in range(N_CORES):
        b, g = divmod(c, 4)
        cols = slice(g * GD, (g + 1) * GD)
        in_maps.append({
            "xq": bf3(query[b]),
            "xk": bf3(key[b]),
            "xv": bf3(value[b]),
            "wq": bf(Wq[:, cols]),
            "wk": bf(Wk[:, cols]),
            "wv": bf(Wv[:, cols]),
            "bqbk_pk": f32(np.tile(np.concatenate(
                [np.asarray(bq)[cols].reshape(HPG, DK).T,
                 np.asarray(bk)[cols].reshape(HPG, DK).T], axis=1), (2, 1))),
            "bv_r": bf(np.asarray(bv)[cols].reshape(1, GD)),
            "mask_pk": f32(np.asarray(mask)[b].reshape(NKC, 128).T),
        })
    return in_maps


def kernel(query, key, value, mask, Wq, bq, Wk, bk, Wv, bv):
    query = np.asarray(query, np.float32)
    key = np.asarray(key, np.float32)
    value = np.asarray(value, np.float32)
    nc = _get_nc()
    in_maps = make_in_maps(query, key, value, mask, Wq, bq, Wk, bk, Wv, bv)
    res = run_bass_kernel_spmd(nc, in_maps, core_ids=list(range(N_CORES)))
    out = np.empty((B, S, D), np.float32)
    for c in range(N_CORES):
        b, g = divmod(c, 4)
        out[b, :, g * GD : (g + 1) * GD] = res.results[c]["out"]
    return out



# revision 11
# speedup vs baseline: 1.0834x; 1.0834x over previous
"""Multi-head attention (B=2, S=4096, D=768, H=12) on 8 Trainium2 cores.

Sharding: core c -> batch b = c // 4, head-triple g = c % 4 (heads 3g..3g+2).
Each core computes its QKV projections (columns of W for its heads) and
flash-style attention for its 3 heads, fully on-chip; no cross-core comms.
Host-side prep per core: slice batch/head-group, cast x/W to fp16 (the device
kernel computes in fp16 with f32 accumulation; fp16 over bf16 because all
value ranges here are small, giving ~8x lower quantization error at identical
PE throughput; casting host-side also halves the transfer volume).

Per-core device kernel:
  - x^T tiles [128(d), 6(dchunk), 1024(s)] via xbar DMA-transpose straight
    from the fp16 DRAM inputs, quarter by quarter; projections chase each
    quarter so ScalarE attention work starts ~tens of us in.
  - projections on PE produce qT/kT [64, 3, 4096] and v_ext
    [128, 32, 3, 65] (col 64 = ones so the PV matmul accumulates the softmax
    denominator as output row 64). The attention mask enters as a per-k scale
    em = exp(-1e4*(1-mask)) folded into v_ext (exact: softmax with additive
    -1e4 adder == scaling exp(s) by em, including the denominator).
  - attention per (head, 512-wide q chunk): 32 k-chunks in groups of 3;
    QK^T -> PSUM, exp on ScalarE (scale=1/8) -> fp16 probs in SBUF,
    PV accumulate -> PSUM [65, 512]; then PE-transpose to natural layout and
    DVE normalize by the reciprocal of the denominator column (row 64).
"""

import sys

if "/opt/trn_rl_repo" not in sys.path:
    sys.path.insert(0, "/opt/trn_rl_repo")

from contextlib import ExitStack

import ml_dtypes
import numpy as np

import concourse.bass as bass
import concourse.tile as tile
from concourse import bacc, mybir
from concourse.bass_utils import run_bass_kernel_spmd
from concourse.masks import make_identity

F32 = mybir.dt.float32
# fp16 instead of bf16: all on-chip value ranges here are tiny (|x|<6,
# |W|<0.12, probs<8), so fp16's 10 mantissa bits cut quantization error ~4x
# at identical PE throughput (1 cycle/row) and xbar 2-byte transpose support
BF16 = mybir.dt.float16
AF = mybir.ActivationFunctionType
ALU = mybir.AluOpType
BF16_NP = np.float16

B, S, D, H, DK = 2, 4096, 768, 12, 64
N_CORES = 8
HPG = 3            # heads per core
GD = HPG * DK      # 192 output columns per core
SQ = 512           # q-chunk width
NSQ = S // SQ      # 8
KCW = 128          # k-chunk width
NKC = S // KCW     # 32
GRP = 3            # k-chunks per exp group (3 PSUM banks, double buffered)
NDC = D // 128     # 6 contraction chunks
QTR = S // 4       # transpose/projection pipeline granularity
SQQ = NSQ // 4     # q chunks per quarter
SCQ = NKC // 4     # s chunks per quarter


def _emit(ctx: ExitStack, tc: tile.TileContext, io: dict):
    nc = tc.nc

    const = ctx.enter_context(tc.tile_pool(name="const", bufs=1))
    xt_pool = ctx.enter_context(tc.tile_pool(name="xt", bufs=5))
    proj = ctx.enter_context(tc.tile_pool(name="proj", bufs=1))
    scores_pool = ctx.enter_context(tc.tile_pool(name="scores", bufs=2, space="PSUM"))
    aux_psum = ctx.enter_context(tc.tile_pool(name="auxp", bufs=2, space="PSUM"))
    probs_pool = ctx.enter_context(tc.tile_pool(name="probs", bufs=5))
    outt_pool = ctx.enter_context(tc.tile_pool(name="outt", bufs=2))
    small = ctx.enter_context(tc.tile_pool(name="small", bufs=2))
    oslab_pool = ctx.enter_context(tc.tile_pool(name="oslab", bufs=3))
    # DVE-exp offload scratch: x tile, Horner ping-pong, and probs output
    dx_pool = ctx.enter_context(tc.tile_pool(name="dx", bufs=2))
    dh_pool = ctx.enter_context(tc.tile_pool(name="dh", bufs=3))
    prd_pool = ctx.enter_context(tc.tile_pool(name="prd", bufs=2))

    # ---- constants / small inputs (consolidated to limit 4KB slot padding) ----
    # mask -> per-k scale em = exp(-1e4 * (1 - mask)), [128, 32] (p, kchunk).
    # Emitted FIRST so the ACT exp-table load lands at the head of the queues.
    mask_em = const.tile([128, 65], F32, name="mask_em")
    mask_t = mask_em[:, 0:32]
    em_sb = mask_em[:, 32:64]
    neg1e4 = mask_em[:, 64:65]
    nc.gpsimd.memset(neg1e4, -10000.0)
    nc.scalar.dma_start(mask_t, io["mask_pk"][:])
    nc.scalar.activation(em_sb, mask_t, AF.Exp, scale=10000.0, bias=neg1e4)

    # weights loaded contiguously (q | k | v along free dim)
    w_all = const.tile([128, NDC, 3 * GD], BF16, name="w_all")
    for i, nm in ((1, "wk"), (0, "wq"), (2, "wv")):
        nc.scalar.dma_start(
            w_all[:, :, i * GD : (i + 1) * GD],
            io[nm].rearrange("(dc p) n -> p dc n", p=128),
        )
    wv_sb = w_all[:, :, 2 * GD : 3 * GD]

    # q/k weights with each head's 64 columns duplicated (projection then
    # replicates qT/kT on both partition halves at no extra PE cost)
    w_dup = const.tile([128, NDC, 2, HPG, 128], BF16, name="w_dup")
    for i in (1, 0):
        for h in range(HPG):
            for rep in range(2):
                nc.vector.tensor_copy(
                    w_dup[:, :, i, h, rep * DK : (rep + 1) * DK],
                    w_all[:, :, i * GD + h * DK : i * GD + (h + 1) * DK],
                )

    bqbk = const.tile([128, 2 * HPG], F32, name="bqbk")
    nc.scalar.dma_start(bqbk[:], io["bqbk_pk"][:])

    bfpack = const.tile([1, 320], BF16, name="bfpack")
    nc.gpsimd.memset(bfpack[:, 0:128], 1.0)
    nc.scalar.dma_start(bfpack[:, 128 : 128 + GD], io["bv_r"][:])
    ones_row = bfpack[:, 0:128]
    bv_sb = bfpack[:, 128 : 128 + GD]

    ident = const.tile([128, 128], F32, name="ident")
    make_identity(nc, ident[:])

    # ---- persistent projection outputs (qT/kT replicated on both halves) ----
    qT = proj.tile([128, HPG, S], BF16, name="qT")
    kT = proj.tile([128, HPG, S], BF16, name="kT")
    vE = proj.tile([128, NKC, HPG, DK + 1], BF16, name="vE")
    nc.gpsimd.memset(vE[:], 1.0)  # ones col 64; data cols overwritten below

    # ---- per-quarter: transpose + project ----
    def load_xt_quarter(nm, qq):
        # host supplies x d-chunk-major [6*4096, 128] so each xbar transpose
        # reads a fully contiguous [1024, 128] block
        xt = xt_pool.tile([128, NDC, QTR], BF16, tag="xt", name=f"xt_{nm}_{qq}")
        for dc in range(NDC):
            base = dc * S + qq * QTR
            nc.sync.dma_start(
                out=xt[:, dc, :], in_=io[nm][base : base + QTR, :],
                transpose=True,
            )
        return xt

    def proj_qk(xt, qq, wi, bias, dst):
        for h in range(HPG):
            for sqq in range(SQQ):
                sq = qq * SQQ + sqq
                ps = aux_psum.tile([128, SQ], F32, tag="aux", name=f"ps_{qq}_{h}_{sqq}")
                for dc in range(NDC):
                    nc.tensor.matmul(
                        ps[:],
                        lhsT=w_dup[:, dc, wi, h, :],
                        rhs=xt[:, dc, sqq * SQ : (sqq + 1) * SQ],
                        start=(dc == 0),
                        stop=(dc == NDC - 1),
                    )
                nc.vector.tensor_scalar(
                    dst[:, h, sq * SQ : (sq + 1) * SQ], ps[:],
                    bias[:, h : h + 1], None, ALU.add,
                )

    def proj_v(xt, qq):
        for scq in range(SCQ):
            sc = qq * SCQ + scq
            ps = aux_psum.tile([128, GD], F32, tag="aux", name=f"psv_{qq}_{scq}")
            for dc in range(NDC):
                nc.tensor.matmul(
                    ps[:],
                    lhsT=xt[:, dc, scq * 128 : (scq + 1) * 128],
                    rhs=wv_sb[:, dc, :],
                    start=(dc == 0),
                    stop=False,
                )
            nc.tensor.matmul(
                ps[:], lhsT=ones_row[:, 0:128], rhs=bv_sb[:], start=False, stop=True
            )
            for h in range(HPG):
                nc.vector.tensor_copy(
                    vE[:, sc, h, 0:DK], ps[:, h * DK : (h + 1) * DK]
                )
            # fold mask scale into v and the denominator ones column
            nc.vector.tensor_scalar(
                vE[:, sc, :, :], vE[:, sc, :, :], em_sb[:, sc : sc + 1], None,
                ALU.mult,
            )

    def proj_kv_quarter(qq):
        xt_k = load_xt_quarter("xk", qq)
        proj_qk(xt_k, qq, 1, bqbk[:, HPG : 2 * HPG], kT)
        xt_v = load_xt_quarter("xv", qq)
        proj_v(xt_v, qq)

    def proj_q_group(xt, qq, h, sqq):
        # borrows a scores-pool slot: the aux pool's two slots hold live pv
        # accumulators / tr tiles during attention (a third tenant would
        # deadlock the in-order PE queue on slot reuse)
        sq = qq * SQQ + sqq
        ps = scores_pool.tile([128, SQ], F32, tag="scores", name=f"psq_{qq}_{h}_{sqq}")
        for dc in range(NDC):
            nc.tensor.matmul(
                ps[:],
                lhsT=w_dup[:, dc, 0, h, :],
                rhs=xt[:, dc, sqq * SQ : (sqq + 1) * SQ],
                start=(dc == 0),
                stop=(dc == NDC - 1),
            )
        nc.vector.tensor_scalar(
            qT[:, h, sq * SQ : (sq + 1) * SQ], ps[:],
            bqbk[:, h : h + 1], None, ALU.add,
        )

    # ---- attention ----
    groups = []
    g0 = 0
    while g0 < NKC:
        groups.append((g0, min(GRP, NKC - g0)))
        g0 += GRP

    # ScalarE exp is the kernel's critical engine (~375us of the wall); shift
    # the last 3 k-chunks of each iteration (except the first) to a DVE
    # degree-6 Horner polynomial for exp(s/8) on [-2.3, 2.3] (max rel err
    # ~9e-4 exact, ~2.6e-3 with fp16 intermediates, on 3/32 of the keys).
    # A/B on HW: the DVE exp-poly offload measured slower (368us vs 299us)
    # than leaving all exp on ScalarE once the finalize split landed, so it
    # is off by default.
    USE_DVE_EXP = __import__("os").environ.get("BASS_DVE_EXP", "0") == "1"
    DVE_KC = (29, 30, 31)
    C_EXP = (1.000824060890309, 0.9996469799097945, 0.49552294276165537,
             0.16473947810651934, 0.04418772554978006, 0.009769214318998872,
             0.001175456119703543)

    def emit_dve_qk_x(h, sq, it):
        # QK for the offloaded chunks + PSUM->fp16 x = s/8 (frees the PSUM
        # bank after one DVE pass). Emitted at the END of the previous
        # iteration so the DVE drains the bank before ACT groups need it.
        sc_d = scores_pool.tile(
            [128, GRP * SQ], F32, tag="scores", name=f"scd_{it}"
        )
        for j, kc in enumerate(DVE_KC):
            ho = 64 * (kc % 2)
            nc.tensor.matmul(
                sc_d[:, j * SQ : (j + 1) * SQ],
                lhsT=kT[ho : ho + DK, h, kc * KCW : (kc + 1) * KCW],
                rhs=qT[ho : ho + DK, h, sq * SQ : (sq + 1) * SQ],
                start=True,
                stop=True,
            )
        xD = dx_pool.tile([128, GRP * SQ], BF16, tag="dx", name=f"dx_{it}")
        nc.vector.tensor_scalar(xD[:], sc_d[:], 0.125, None, ALU.mult)
        prD = prd_pool.tile([128, GRP * SQ], BF16, tag="prd", name=f"prd_{it}")
        return xD, prD

    def emit_dve_poly(xD, prD, it):
        # Horner: h = c6*x + c5; h = h*x + c4; ...; pr = h*x + c0
        h0 = dh_pool.tile([128, GRP * SQ], BF16, tag="dh", name=f"dh0_{it}")
        nc.vector.tensor_scalar(h0[:], xD[:], C_EXP[6], C_EXP[5], ALU.mult, ALU.add)
        cur = h0
        for step, ck in enumerate((C_EXP[4], C_EXP[3], C_EXP[2], C_EXP[1])):
            m = dh_pool.tile([128, GRP * SQ], BF16, tag="dh", name=f"dhm{step}_{it}")
            nc.vector.tensor_tensor(m[:], cur[:], xD[:], ALU.mult)
            a = dh_pool.tile([128, GRP * SQ], BF16, tag="dh", name=f"dha{step}_{it}")
            nc.vector.tensor_scalar(a[:], m[:], ck, None, ALU.add)
            cur = a
        m = dh_pool.tile([128, GRP * SQ], BF16, tag="dh", name=f"dhm4_{it}")
        nc.vector.tensor_tensor(m[:], cur[:], xD[:], ALU.mult)
        nc.vector.tensor_scalar(prD[:], m[:], C_EXP[0], None, ALU.add)

    pending = None  # finalize closure for the previous (h, sq)

    # finalize split in two: part 1 (the pv->SBUF copy, which frees the pv
    # PSUM slot) fires at gi==0 so it lands in the DVE queue BEFORE the
    # ~10us exp-poly chain; part 2 (PE transposes + normalize + store) at
    # gi==1 so the transposes sit behind QK g1 in the PE queue and never
    # stall on the copy.
    def make_fin_copy(pv, h, sq):
        ot = outt_pool.tile([DK + 1, SQ], F32, tag="outt", name=f"ot_{h}_{sq}")

        def fin1():
            nc.vector.tensor_copy(ot[:], pv[:])
        return ot, fin1

    def make_finalize(ot, h, sq):
        def fin():
            tr = aux_psum.tile([128, 4 * (DK + 1)], F32, tag="aux", name=f"tr_{h}_{sq}")
            for t in range(4):
                nc.tensor.transpose(
                    tr[:, t * (DK + 1) : (t + 1) * (DK + 1)],
                    ot[:, t * 128 : (t + 1) * 128],
                    ident[0 : DK + 1, 0 : DK + 1],
                )
            rc = small.tile([128, 4], F32, tag="recip", name=f"rc_{h}_{sq}")
            osl = oslab_pool.tile([128, 4, DK], F32, tag="oslab", name=f"os_{h}_{sq}")
            for t in range(4):
                nc.vector.reciprocal(
                    rc[:, t : t + 1], tr[:, t * (DK + 1) + DK : t * (DK + 1) + DK + 1]
                )
                nc.vector.tensor_scalar(
                    osl[:, t, :],
                    tr[:, t * (DK + 1) : t * (DK + 1) + DK],
                    rc[:, t : t + 1],
                    None,
                    ALU.mult,
                )
            nc.gpsimd.dma_start(
                out=io["out"].rearrange(
                    "(sq t p) n -> sq p t n", sq=NSQ, t=4, p=128
                )[sq, :, :, h * DK : (h + 1) * DK],
                in_=osl[:],
            )
        return fin

    # Boundary tasks: q-quarter transposes + projection groups for sq>=2 are
    # deferred into the attention phase (PE has per-group slack there), keyed
    # by the flat iteration index after which they are emitted.
    boundary_tasks = {}
    xt_q_tiles = {}

    def sched(it, fn):
        boundary_tasks.setdefault(it, []).append(fn)

    def tr_q(qq):
        def fn():
            xt_q_tiles[qq] = load_xt_quarter("xq", qq)
        return fn

    def pg(qq, h, sqq):
        def fn():
            proj_q_group(xt_q_tiles[qq], qq, h, sqq)
        return fn

    # task scheduled at boundary b fires during iteration b+1 (after its first
    # exp group), so pg for (h, sq) must sit at boundary <= idx(h, sq) - 2
    DEFER_PG = __import__("os").environ.get("BASS_DEFER_PG", "0") == "1"
    if DEFER_PG:
        sched(0, pg(1, 0, 0))
        sched(1, pg(1, 0, 1))
        sched(2, pg(2, 0, 0))
        sched(3, pg(2, 0, 1))
        sched(4, pg(3, 0, 0))
        sched(5, pg(3, 0, 1))
        nb = 6
        for h in (1, 2):
            for qq in (1, 2, 3):
                for sqq in range(SQQ):
                    sched(nb, pg(qq, h, sqq))
                    nb += 1

    def attention_gen():
        nonlocal_pending = [None]

        def emit_pv(pv, h, grp):
            p0, plen, ppr = grp
            for j in range(plen):
                kc = p0 + j
                nc.tensor.matmul(
                    pv[:],
                    lhsT=vE[:, kc, h, :],
                    rhs=ppr[:, j * SQ : (j + 1) * SQ],
                    start=(kc == 0),
                    stop=(kc == NKC - 1),
                )

        carry = None  # (pv, h, [groups]) tail-PV work carried across iterations
        it = 0
        iters = [(h, sq) for h in range(HPG) for sq in range(NSQ)]
        pending_dve = None  # (xD, prD) prepared for the iteration about to run
        for h, sq in iters:
            dve_state, pending_dve = pending_dve, None
            if dve_state is not None:
                act_groups = [(g, GRP) for g in range(0, 27, GRP)] + [(27, 2)]
            else:
                act_groups = groups
            pv = aux_psum.tile([DK + 1, SQ], F32, tag="aux", name=f"pv_{h}_{sq}")
            ready = []  # (kc0, glen, probs) groups awaiting PV emission
            for gi, (kc0, glen) in enumerate(act_groups):
                sc = scores_pool.tile(
                    [128, glen * SQ], F32, tag="scores",
                    name=f"sc_{h}_{sq}_{gi}",
                )
                for j in range(glen):
                    kc = kc0 + j
                    # alternate PE row groups by kc parity: qT/kT are
                    # replicated on partitions 64-127, so odd chunks read
                    # the upper half and land on row tiles (64,0) — the PE
                    # runs the two K=64 matmuls concurrently (row tiling)
                    ho = 64 * (kc % 2)
                    nc.tensor.matmul(
                        sc[:, j * SQ : (j + 1) * SQ],
                        lhsT=kT[ho : ho + DK, h, kc * KCW : (kc + 1) * KCW],
                        rhs=qT[ho : ho + DK, h, sq * SQ : (sq + 1) * SQ],
                        start=True,
                        stop=True,
                    )
                pr = probs_pool.tile(
                    [128, glen * SQ], BF16, tag="probs",
                    name=f"pr_{h}_{sq}_{gi}",
                )
                nc.scalar.activation(pr[:], sc[:], AF.Exp, scale=0.125)
                ready.append((kc0, glen, pr))
                if gi == 0:
                    if carry is not None:
                        cpv, ch, cgrps = carry
                        for grp in cgrps:
                            emit_pv(cpv, ch, grp)
                        carry = None
                        for fn in boundary_tasks.get(it - 1, ()):
                            fn()
                    # free the previous pv PSUM slot (DVE copy) BEFORE the
                    # poly chain enters the DVE queue
                    if nonlocal_pending[0] is not None:
                        nonlocal_pending[0][0]()
                    if dve_state is not None:
                        emit_dve_poly(dve_state[0], dve_state[1], it)
                if gi == 1 and nonlocal_pending[0] is not None:
                    nonlocal_pending[0][1]()
                    nonlocal_pending[0] = None
                if len(ready) >= 2:
                    emit_pv(pv, h, ready.pop(0))
                yield (h, sq, gi)
            if USE_DVE_EXP and it + 1 < len(iters):
                nh, nsq = iters[it + 1]
                pending_dve = emit_dve_qk_x(nh, nsq, it + 1)
            carry_grps = list(ready)
            if dve_state is not None:
                carry_grps.append((DVE_KC[0], len(DVE_KC), dve_state[1]))
            carry = (pv, h, carry_grps)
            ot, fin1 = make_fin_copy(pv, h, sq)
            nonlocal_pending[0] = (fin1, make_finalize(ot, h, sq))
            it += 1

        cpv, ch, cgrps = carry
        for grp in cgrps:
            emit_pv(cpv, ch, grp)
        nonlocal_pending[0][0]()
        nonlocal_pending[0][1]()

    # Interleave k/v projection quarters with the first attention iteration's
    # k-chunk groups so ScalarE saturates early: group gi covers kc
    # 3*gi..3*gi+2, requiring k/v quarters up to (3*gi+2)//8; the first
    # iteration uses q chunk sq=0 (quarter 0).
    gen = attention_gen()

    def advance(n):
        for _ in range(n):
            if next(gen, None) is None:
                break

    # quarter 0 in k, q, v order: the first QK group needs kT+qT only (the
    # first PV trails by two exp groups, so v can land a little later)
    xt_k0 = load_xt_quarter("xk", 0)
    proj_qk(xt_k0, 0, 1, bqbk[:, HPG : 2 * HPG], kT)
    xt_q_tiles[0] = load_xt_quarter("xq", 0)
    for h in range(HPG):
        for sqq in range(SQQ):
            proj_q_group(xt_q_tiles[0], 0, h, sqq)
    advance(1)       # g0: kc 0..2 (needs only kT+qT of quarter 0)
    xt_v0 = load_xt_quarter("xv", 0)
    proj_v(xt_v0, 0)
    advance(1)       # g1: kc 3..5 (first PV fires after this exp)
    proj_kv_quarter(1)
    xt_q_tiles[1] = load_xt_quarter("xq", 1)
    if not DEFER_PG:
        for h in range(HPG):
            for sqq in range(SQQ):
                proj_q_group(xt_q_tiles[1], 1, h, sqq)
    advance(3)       # g2..g4: kc 6..14 (quarters 0-1)
    proj_kv_quarter(2)
    xt_q_tiles[2] = load_xt_quarter("xq", 2)
    if not DEFER_PG:
        for h in range(HPG):
            for sqq in range(SQQ):
                proj_q_group(xt_q_tiles[2], 2, h, sqq)
    advance(3)       # g5..g7: kc 15..23 (quarter 2)
    proj_kv_quarter(3)
    xt_q_tiles[3] = load_xt_quarter("xq", 3)
    if not DEFER_PG:
        for h in range(HPG):
            for sqq in range(SQQ):
                proj_q_group(xt_q_tiles[3], 3, h, sqq)
    for _ in gen:
        pass


def _build():
    nc = bacc.Bacc("TRN2", target_bir_lowering=False, debug=False)
    io = {}
    for nm, shape, dt in (
        ("xq", [NDC * S, 128], BF16), ("xk", [NDC * S, 128], BF16),
        ("xv", [NDC * S, 128], BF16),
        ("wq", [D, GD], BF16), ("wk", [D, GD], BF16), ("wv", [D, GD], BF16),
        ("bqbk_pk", [128, 2 * HPG], F32),
        ("bv_r", [1, GD], BF16), ("mask_pk", [128, NKC], F32),
    ):
        io[nm] = nc.dram_tensor(nm, shape, dt, kind="ExternalInput").ap()
    io["out"] = nc.dram_tensor("out", [S, GD], F32, kind="ExternalOutput").ap()

    import os

    dup = int(os.environ.get("BASS_DUP", "1"))
    with tile.TileContext(nc) as tc:
        for _ in range(dup):
            with ExitStack() as ctx:
                _emit(ctx, tc, io)
    nc.compile()
    return nc


_NC = None


def _get_nc():
    global _NC
    if _NC is None:
        _NC = _build()
    return _NC


def make_in_maps(query, key, value, mask, Wq, bq, Wk, bk, Wv, bv):
    bf = lambda a: np.ascontiguousarray(a).astype(BF16_NP)
    bf3 = lambda a: np.ascontiguousarray(
        np.asarray(a).reshape(S, NDC, 128).transpose(1, 0, 2).reshape(NDC * S, 128)
    ).astype(BF16_NP)
    f32 = lambda a: np.ascontiguousarray(np.asarray(a, np.float32))
    in_maps = []
    for c in range(N_CORES):
        b, g = divmod(c, 4)
        cols = slice(g * GD, (g + 1) * GD)
        in_maps.append({
            "xq": bf3(query[b]),
            "xk": bf3(key[b]),
            "xv": bf3(value[b]),
            "wq": bf(Wq[:, cols]),
            "wk": bf(Wk[:, cols]),
            "wv": bf(Wv[:, cols]),
            "bqbk_pk": f32(np.tile(np.concatenate(
                [np.asarray(bq)[cols].reshape(HPG, DK).T,
                 np.asarray(bk)[cols].reshape(HPG, DK).T], axis=1), (2, 1))),
            "bv_r": bf(np.asarray(bv)[cols].reshape(1, GD)),
            "mask_pk": f32(np.asarray(mask)[b].reshape(NKC, 128).T),
        })
    return in_maps


def kernel(query, key, value, mask, Wq, bq, Wk, bk, Wv, bv):
    query = np.asarray(query, np.float32)
    key = np.asarray(key, np.float32)
    value = np.asarray(value, np.float32)
    nc = _get_nc()
    in_maps = make_in_maps(query, key, value, mask, Wq, bq, Wk, bk, Wv, bv)
    res = run_bass_kernel_spmd(nc, in_maps, core_ids=list(range(N_CORES)))
    out = np.empty((B, S, D), np.float32)
    for c in range(N_CORES):
        b, g = divmod(c, 4)
        out[b, :, g * GD : (g + 1) * GD] = res.results[c]["out"]
    return out



# revision 18
# speedup vs baseline: 1.0850x; 1.0015x over previous
"""Multi-head attention (B=2, S=4096, D=768, H=12) on 8 Trainium2 cores.

Sharding: core c -> batch b = c // 4, head-triple g = c % 4 (heads 3g..3g+2).
Each core computes its QKV projections (columns of W for its heads) and
flash-style attention for its 3 heads, fully on-chip; no cross-core comms.
Host-side prep per core: slice batch/head-group, cast x/W to fp16 (the device
kernel computes in fp16 with f32 accumulation; fp16 over bf16 because all
value ranges here are small, giving ~8x lower quantization error at identical
PE throughput; casting host-side also halves the transfer volume).

Per-core device kernel:
  - x^T tiles [128(d), 6(dchunk), 1024(s)] via xbar DMA-transpose straight
    from the fp16 DRAM inputs, quarter by quarter; projections chase each
    quarter so ScalarE attention work starts ~tens of us in.
  - projections on PE produce qT/kT [64, 3, 4096] and v_ext
    [128, 32, 3, 65] (col 64 = ones so the PV matmul accumulates the softmax
    denominator as output row 64). The attention mask enters as a per-k scale
    em = exp(-1e4*(1-mask)) folded into v_ext (exact: softmax with additive
    -1e4 adder == scaling exp(s) by em, including the denominator).
  - attention per (head, 512-wide q chunk): 32 k-chunks in groups of 3;
    QK^T -> PSUM, exp on ScalarE (scale=1/8) -> fp16 probs in SBUF,
    PV accumulate -> PSUM [65, 512]; then PE-transpose to natural layout and
    DVE normalize by the reciprocal of the denominator column (row 64).

Perf structure (HW-verified this series; engine rates from microbenchmarks in
probe.py):
  - QK^T row tiling: odd k-chunks read the qT/kT replicas on partitions
    64-127, so consecutive K=64 matmuls land on different PE row groups and
    run concurrently (589us -> 388us like-for-like). This is the reason for
    the w_dup duplication.
  - split finalize: the pv->SBUF copy is emitted at the NEXT iteration's
    first exp group (ahead of everything else DVE does that iteration) so
    the pv PSUM slot frees early; transposes+normalize one group later so
    they queue behind QK g1 on the PE.
  - fast-start prologue: only the (h0,s0) k+q projections gate the first
    exp group.
  - rejected on HW A/B (kept opt-in): BASS_DVE_EXP=1 offloads 3 k-chunks/
    iter of exp to a DVE deg-6 Horner poly (368us vs 299us best-case);
    BASS_PAIR=1 pairs QK emission across exp-group boundaries (478 vs 450).
  - PSUM budget: scores 2 bufs x 3 banks + pv 1 + tr 1 = 8 (full). GRP=4
    exp groups or 3-deep scores buffering require freeing a bank first.
"""

import sys

if "/opt/trn_rl_repo" not in sys.path:
    sys.path.insert(0, "/opt/trn_rl_repo")

from contextlib import ExitStack

import ml_dtypes
import numpy as np

import concourse.bass as bass
import concourse.tile as tile
from concourse import bacc, mybir
from concourse.bass_utils import run_bass_kernel_spmd
from concourse.masks import make_identity

F32 = mybir.dt.float32
# fp16 instead of bf16: all on-chip value ranges here are tiny (|x|<6,
# |W|<0.12, probs<8), so fp16's 10 mantissa bits cut quantization error ~4x
# at identical PE throughput (1 cycle/row) and xbar 2-byte transpose support
BF16 = mybir.dt.float16
AF = mybir.ActivationFunctionType
ALU = mybir.AluOpType
BF16_NP = np.float16

B, S, D, H, DK = 2, 4096, 768, 12, 64
N_CORES = 8
HPG = 3            # heads per core
GD = HPG * DK      # 192 output columns per core
SQ = 512           # q-chunk width
NSQ = S // SQ      # 8
KCW = 128          # k-chunk width
NKC = S // KCW     # 32
GRP = 3            # k-chunks per exp group (3 PSUM banks, double buffered)
NDC = D // 128     # 6 contraction chunks
QTR = S // 4       # transpose/projection pipeline granularity
SQQ = NSQ // 4     # q chunks per quarter
SCQ = NKC // 4     # s chunks per quarter


def _emit(ctx: ExitStack, tc: tile.TileContext, io: dict):
    nc = tc.nc

    const = ctx.enter_context(tc.tile_pool(name="const", bufs=1))
    xt_pool = ctx.enter_context(tc.tile_pool(name="xt", bufs=5))
    proj = ctx.enter_context(tc.tile_pool(name="proj", bufs=1))
    scores_pool = ctx.enter_context(tc.tile_pool(name="scores", bufs=2, space="PSUM"))
    aux_psum = ctx.enter_context(tc.tile_pool(name="auxp", bufs=2, space="PSUM"))
    probs_pool = ctx.enter_context(tc.tile_pool(name="probs", bufs=5))
    outt_pool = ctx.enter_context(tc.tile_pool(name="outt", bufs=2))
    small = ctx.enter_context(tc.tile_pool(name="small", bufs=2))
    oslab_pool = ctx.enter_context(tc.tile_pool(name="oslab", bufs=3))
    # DVE-exp offload scratch: x tile, Horner ping-pong, and probs output
    dx_pool = ctx.enter_context(tc.tile_pool(name="dx", bufs=2))
    dh_pool = ctx.enter_context(tc.tile_pool(name="dh", bufs=3))
    prd_pool = ctx.enter_context(tc.tile_pool(name="prd", bufs=2))

    # ---- constants / small inputs (consolidated to limit 4KB slot padding) ----
    # mask -> per-k scale em = exp(-1e4 * (1 - mask)), [128, 32] (p, kchunk).
    # Emitted FIRST so the ACT exp-table load lands at the head of the queues.
    mask_em = const.tile([128, 65], F32, name="mask_em")
    mask_t = mask_em[:, 0:32]
    em_sb = mask_em[:, 32:64]
    neg1e4 = mask_em[:, 64:65]
    nc.gpsimd.memset(neg1e4, -10000.0)
    nc.scalar.dma_start(mask_t, io["mask_pk"][:])
    nc.scalar.activation(em_sb, mask_t, AF.Exp, scale=10000.0, bias=neg1e4)

    # weights loaded contiguously (q | k | v along free dim)
    w_all = const.tile([128, NDC, 3 * GD], BF16, name="w_all")
    for i, nm in ((1, "wk"), (0, "wq"), (2, "wv")):
        nc.scalar.dma_start(
            w_all[:, :, i * GD : (i + 1) * GD],
            io[nm].rearrange("(dc p) n -> p dc n", p=128),
        )
    wv_sb = w_all[:, :, 2 * GD : 3 * GD]

    # q/k weights with each head's 64 columns duplicated (projection then
    # replicates qT/kT on both partition halves at no extra PE cost)
    w_dup = const.tile([128, NDC, 2, HPG, 128], BF16, name="w_dup")
    for i in (1, 0):
        for h in range(HPG):
            for rep in range(2):
                nc.vector.tensor_copy(
                    w_dup[:, :, i, h, rep * DK : (rep + 1) * DK],
                    w_all[:, :, i * GD + h * DK : i * GD + (h + 1) * DK],
                )

    bqbk = const.tile([128, 2 * HPG], F32, name="bqbk")
    nc.scalar.dma_start(bqbk[:], io["bqbk_pk"][:])

    bfpack = const.tile([1, 320], BF16, name="bfpack")
    nc.gpsimd.memset(bfpack[:, 0:128], 1.0)
    nc.scalar.dma_start(bfpack[:, 128 : 128 + GD], io["bv_r"][:])
    ones_row = bfpack[:, 0:128]
    bv_sb = bfpack[:, 128 : 128 + GD]

    ident = const.tile([128, 128], F32, name="ident")
    make_identity(nc, ident[:])

    # ---- persistent projection outputs (qT/kT replicated on both halves) ----
    qT = proj.tile([128, HPG, S], BF16, name="qT")
    kT = proj.tile([128, HPG, S], BF16, name="kT")
    vE = proj.tile([128, NKC, HPG, DK + 1], BF16, name="vE")
    nc.gpsimd.memset(vE[:], 1.0)  # ones col 64; data cols overwritten below

    # ---- per-quarter: transpose + project ----
    def load_xt_quarter(nm, qq):
        # host supplies x d-chunk-major [6*4096, 128] so each xbar transpose
        # reads a fully contiguous [1024, 128] block
        xt = xt_pool.tile([128, NDC, QTR], BF16, tag="xt", name=f"xt_{nm}_{qq}")
        for dc in range(NDC):
            base = dc * S + qq * QTR
            nc.sync.dma_start(
                out=xt[:, dc, :], in_=io[nm][base : base + QTR, :],
                transpose=True,
            )
        return xt

    def proj_qk_one(xt, qq, wi, bias, dst, h, sqq):
        sq = qq * SQQ + sqq
        ps = aux_psum.tile([128, SQ], F32, tag="aux", name=f"ps_{qq}_{h}_{sqq}")
        for dc in range(NDC):
            nc.tensor.matmul(
                ps[:],
                lhsT=w_dup[:, dc, wi, h, :],
                rhs=xt[:, dc, sqq * SQ : (sqq + 1) * SQ],
                start=(dc == 0),
                stop=(dc == NDC - 1),
            )
        nc.vector.tensor_scalar(
            dst[:, h, sq * SQ : (sq + 1) * SQ], ps[:],
            bias[:, h : h + 1], None, ALU.add,
        )

    def proj_qk(xt, qq, wi, bias, dst, skip=None):
        for h in range(HPG):
            for sqq in range(SQQ):
                if skip is not None and (h, sqq) in skip:
                    continue
                proj_qk_one(xt, qq, wi, bias, dst, h, sqq)

    def proj_v(xt, qq):
        for scq in range(SCQ):
            sc = qq * SCQ + scq
            ps = aux_psum.tile([128, GD], F32, tag="aux", name=f"psv_{qq}_{scq}")
            for dc in range(NDC):
                nc.tensor.matmul(
                    ps[:],
                    lhsT=xt[:, dc, scq * 128 : (scq + 1) * 128],
                    rhs=wv_sb[:, dc, :],
                    start=(dc == 0),
                    stop=False,
                )
            nc.tensor.matmul(
                ps[:], lhsT=ones_row[:, 0:128], rhs=bv_sb[:], start=False, stop=True
            )
            for h in range(HPG):
                nc.vector.tensor_copy(
                    vE[:, sc, h, 0:DK], ps[:, h * DK : (h + 1) * DK]
                )
            # fold mask scale into v and the denominator ones column
            nc.vector.tensor_scalar(
                vE[:, sc, :, :], vE[:, sc, :, :], em_sb[:, sc : sc + 1], None,
                ALU.mult,
            )

    def proj_kv_quarter(qq):
        xt_k = load_xt_quarter("xk", qq)
        proj_qk(xt_k, qq, 1, bqbk[:, HPG : 2 * HPG], kT)
        xt_v = load_xt_quarter("xv", qq)
        proj_v(xt_v, qq)

    def proj_q_group(xt, qq, h, sqq):
        # borrows a scores-pool slot: the aux pool's two slots hold live pv
        # accumulators / tr tiles during attention (a third tenant would
        # deadlock the in-order PE queue on slot reuse)
        sq = qq * SQQ + sqq
        ps = scores_pool.tile([128, SQ], F32, tag="scores", name=f"psq_{qq}_{h}_{sqq}")
        for dc in range(NDC):
            nc.tensor.matmul(
                ps[:],
                lhsT=w_dup[:, dc, 0, h, :],
                rhs=xt[:, dc, sqq * SQ : (sqq + 1) * SQ],
                start=(dc == 0),
                stop=(dc == NDC - 1),
            )
        nc.vector.tensor_scalar(
            qT[:, h, sq * SQ : (sq + 1) * SQ], ps[:],
            bqbk[:, h : h + 1], None, ALU.add,
        )

    # ---- attention ----
    groups = []
    g0 = 0
    while g0 < NKC:
        groups.append((g0, min(GRP, NKC - g0)))
        g0 += GRP

    # ScalarE exp is the kernel's critical engine (~375us of the wall); shift
    # the last 3 k-chunks of each iteration (except the first) to a DVE
    # degree-6 Horner polynomial for exp(s/8) on [-2.3, 2.3] (max rel err
    # ~9e-4 exact, ~2.6e-3 with fp16 intermediates, on 3/32 of the keys).
    # A/B on HW: the DVE exp-poly offload measured slower (368us vs 299us)
    # than leaving all exp on ScalarE once the finalize split landed, so it
    # is off by default.
    USE_DVE_EXP = __import__("os").environ.get("BASS_DVE_EXP", "0") == "1"
    DVE_KC = (29, 30, 31)
    C_EXP = (1.000824060890309, 0.9996469799097945, 0.49552294276165537,
             0.16473947810651934, 0.04418772554978006, 0.009769214318998872,
             0.001175456119703543)

    def emit_dve_qk_x(h, sq, it):
        # QK for the offloaded chunks + PSUM->fp16 x = s/8 (frees the PSUM
        # bank after one DVE pass). Emitted at the END of the previous
        # iteration so the DVE drains the bank before ACT groups need it.
        sc_d = scores_pool.tile(
            [128, GRP * SQ], F32, tag="scores", name=f"scd_{it}"
        )
        for j, kc in enumerate(DVE_KC):
            ho = 64 * (kc % 2)
            nc.tensor.matmul(
                sc_d[:, j * SQ : (j + 1) * SQ],
                lhsT=kT[ho : ho + DK, h, kc * KCW : (kc + 1) * KCW],
                rhs=qT[ho : ho + DK, h, sq * SQ : (sq + 1) * SQ],
                start=True,
                stop=True,
            )
        xD = dx_pool.tile([128, GRP * SQ], BF16, tag="dx", name=f"dx_{it}")
        nc.vector.tensor_scalar(xD[:], sc_d[:], 0.125, None, ALU.mult)
        prD = prd_pool.tile([128, GRP * SQ], BF16, tag="prd", name=f"prd_{it}")
        return xD, prD

    def emit_dve_poly(xD, prD, it):
        # Horner: h = c6*x + c5; h = h*x + c4; ...; pr = h*x + c0
        h0 = dh_pool.tile([128, GRP * SQ], BF16, tag="dh", name=f"dh0_{it}")
        nc.vector.tensor_scalar(h0[:], xD[:], C_EXP[6], C_EXP[5], ALU.mult, ALU.add)
        cur = h0
        for step, ck in enumerate((C_EXP[4], C_EXP[3], C_EXP[2], C_EXP[1])):
            m = dh_pool.tile([128, GRP * SQ], BF16, tag="dh", name=f"dhm{step}_{it}")
            nc.vector.tensor_tensor(m[:], cur[:], xD[:], ALU.mult)
            a = dh_pool.tile([128, GRP * SQ], BF16, tag="dh", name=f"dha{step}_{it}")
            nc.vector.tensor_scalar(a[:], m[:], ck, None, ALU.add)
            cur = a
        m = dh_pool.tile([128, GRP * SQ], BF16, tag="dh", name=f"dhm4_{it}")
        nc.vector.tensor_tensor(m[:], cur[:], xD[:], ALU.mult)
        nc.vector.tensor_scalar(prD[:], m[:], C_EXP[0], None, ALU.add)

    pending = None  # finalize closure for the previous (h, sq)

    # finalize split in two: part 1 (the pv->SBUF copy, which frees the pv
    # PSUM slot) fires at gi==0 so it lands in the DVE queue BEFORE the
    # ~10us exp-poly chain; part 2 (PE transposes + normalize + store) at
    # gi==1 so the transposes sit behind QK g1 in the PE queue and never
    # stall on the copy.
    def make_fin_copy(pv, h, sq):
        ot = outt_pool.tile([DK + 1, SQ], F32, tag="outt", name=f"ot_{h}_{sq}")

        def fin1():
            nc.vector.tensor_copy(ot[:], pv[:])
        return ot, fin1

    def make_finalize(ot, h, sq):
        def fin():
            tr = aux_psum.tile([128, 4 * (DK + 1)], F32, tag="aux", name=f"tr_{h}_{sq}")
            for t in range(4):
                nc.tensor.transpose(
                    tr[:, t * (DK + 1) : (t + 1) * (DK + 1)],
                    ot[:, t * 128 : (t + 1) * 128],
                    ident[0 : DK + 1, 0 : DK + 1],
                )
            rc = small.tile([128, 4], F32, tag="recip", name=f"rc_{h}_{sq}")
            osl = oslab_pool.tile([128, 4, DK], F32, tag="oslab", name=f"os_{h}_{sq}")
            for t in range(4):
                nc.vector.reciprocal(
                    rc[:, t : t + 1], tr[:, t * (DK + 1) + DK : t * (DK + 1) + DK + 1]
                )
                nc.vector.tensor_scalar(
                    osl[:, t, :],
                    tr[:, t * (DK + 1) : t * (DK + 1) + DK],
                    rc[:, t : t + 1],
                    None,
                    ALU.mult,
                )
            nc.gpsimd.dma_start(
                out=io["out"].rearrange(
                    "(sq t p) n -> sq p t n", sq=NSQ, t=4, p=128
                )[sq, :, :, h * DK : (h + 1) * DK],
                in_=osl[:],
            )
        return fin

    # Boundary tasks: q-quarter transposes + projection groups for sq>=2 are
    # deferred into the attention phase (PE has per-group slack there), keyed
    # by the flat iteration index after which they are emitted.
    boundary_tasks = {}
    xt_q_tiles = {}

    def sched(it, fn):
        boundary_tasks.setdefault(it, []).append(fn)

    def tr_q(qq):
        def fn():
            xt_q_tiles[qq] = load_xt_quarter("xq", qq)
        return fn

    def pg(qq, h, sqq):
        def fn():
            proj_q_group(xt_q_tiles[qq], qq, h, sqq)
        return fn

    # task scheduled at boundary b fires during iteration b+1 (after its first
    # exp group), so pg for (h, sq) must sit at boundary <= idx(h, sq) - 2
    DEFER_PG = __import__("os").environ.get("BASS_DEFER_PG", "0") == "1"
    if DEFER_PG:
        sched(0, pg(1, 0, 0))
        sched(1, pg(1, 0, 1))
        sched(2, pg(2, 0, 0))
        sched(3, pg(2, 0, 1))
        sched(4, pg(3, 0, 0))
        sched(5, pg(3, 0, 1))
        nb = 6
        for h in (1, 2):
            for qq in (1, 2, 3):
                for sqq in range(SQQ):
                    sched(nb, pg(qq, h, sqq))
                    nb += 1

    def attention_gen():
        nonlocal_pending = [None]

        def emit_pv(pv, h, grp):
            p0, plen, ppr = grp
            for j in range(plen):
                kc = p0 + j
                nc.tensor.matmul(
                    pv[:],
                    lhsT=vE[:, kc, h, :],
                    rhs=ppr[:, j * SQ : (j + 1) * SQ],
                    start=(kc == 0),
                    stop=(kc == NKC - 1),
                )

        carry = None  # (pv, h, [groups]) tail-PV work carried across iterations
        it = 0
        iters = [(h, sq) for h in range(HPG) for sq in range(NSQ)]
        pending_dve = None  # (xD, prD) prepared for the iteration about to run
        for h, sq in iters:
            dve_state, pending_dve = pending_dve, None
            if dve_state is not None:
                act_groups = [(g, GRP) for g in range(0, 27, GRP)] + [(27, 2)]
            else:
                act_groups = groups
            pv = aux_psum.tile([DK + 1, SQ], F32, tag="aux", name=f"pv_{h}_{sq}")
            ready = []  # (kc0, glen, probs) groups awaiting PV emission
            # QK matmuls are emitted in strict (even, odd) kc pairs ACROSS
            # group boundaries so every matmul lands adjacent to its
            # opposite-row-group partner in the PE queue and the two K=64
            # halves run concurrently (PV blocks between groups would
            # otherwise orphan each group's 3rd chunk)
            chunk_list = []
            for gi, (kc0, glen) in enumerate(act_groups):
                for j in range(glen):
                    chunk_list.append((kc0 + j, gi, j))
            sc_tiles = {}
            filled = [0] * len(act_groups)
            fired = 0
            ci = 0
            # no pairing in iteration 0: its one-chunk lookahead would hold a
            # live scores tile across the prologue's advance() points, where
            # proj_q_group borrows slots from the same pool
            # A/B on HW: pairing measured neutral-to-worse (478us vs 450us
            # best-valid samples) — likely the 1-chunk lookahead couples the
            # PE to the previous exp via the scores double-buffer. Opt-in.
            do_pair = __import__("os").environ.get("BASS_PAIR", "0") == "1"
            pair_n = 2 if (it > 0 and do_pair) else 1
            while ci < len(chunk_list):
                for _ in range(pair_n):
                    if ci >= len(chunk_list):
                        break
                    kc, gi, j = chunk_list[ci]
                    ci += 1
                    if gi not in sc_tiles:
                        sc_tiles[gi] = scores_pool.tile(
                            [128, act_groups[gi][1] * SQ], F32, tag="scores",
                            name=f"sc_{h}_{sq}_{gi}",
                        )
                    ho = 64 * (kc % 2)
                    nc.tensor.matmul(
                        sc_tiles[gi][:, j * SQ : (j + 1) * SQ],
                        lhsT=kT[ho : ho + DK, h, kc * KCW : (kc + 1) * KCW],
                        rhs=qT[ho : ho + DK, h, sq * SQ : (sq + 1) * SQ],
                        start=True,
                        stop=True,
                    )
                    filled[gi] += 1
                while fired < len(act_groups) and (
                    filled[fired] == act_groups[fired][1]
                ):
                    gi = fired
                    kc0, glen = act_groups[gi]
                    pr = probs_pool.tile(
                        [128, glen * SQ], BF16, tag="probs",
                        name=f"pr_{h}_{sq}_{gi}",
                    )
                    nc.scalar.activation(
                        pr[:], sc_tiles.pop(gi)[:], AF.Exp, scale=0.125
                    )
                    ready.append((kc0, glen, pr))
                    if gi == 0:
                        if carry is not None:
                            cpv, ch, cgrps = carry
                            for grp in cgrps:
                                emit_pv(cpv, ch, grp)
                            carry = None
                            for fn in boundary_tasks.get(it - 1, ()):
                                fn()
                        # free the previous pv PSUM slot (DVE copy) BEFORE
                        # the poly chain enters the DVE queue
                        if nonlocal_pending[0] is not None:
                            nonlocal_pending[0][0]()
                        if dve_state is not None:
                            emit_dve_poly(dve_state[0], dve_state[1], it)
                    if gi == 1 and nonlocal_pending[0] is not None:
                        nonlocal_pending[0][1]()
                        nonlocal_pending[0] = None
                    if len(ready) >= 2:
                        emit_pv(pv, h, ready.pop(0))
                    fired += 1
                    yield (h, sq, gi)
            if USE_DVE_EXP and it + 1 < len(iters):
                nh, nsq = iters[it + 1]
                pending_dve = emit_dve_qk_x(nh, nsq, it + 1)
            carry_grps = list(ready)
            if dve_state is not None:
                carry_grps.append((DVE_KC[0], len(DVE_KC), dve_state[1]))
            carry = (pv, h, carry_grps)
            ot, fin1 = make_fin_copy(pv, h, sq)
            nonlocal_pending[0] = (fin1, make_finalize(ot, h, sq))
            it += 1

        cpv, ch, cgrps = carry
        for grp in cgrps:
            emit_pv(cpv, ch, grp)
        nonlocal_pending[0][0]()
        nonlocal_pending[0][1]()

    # Interleave k/v projection quarters with the first attention iteration's
    # k-chunk groups so ScalarE saturates early: group gi covers kc
    # 3*gi..3*gi+2, requiring k/v quarters up to (3*gi+2)//8; the first
    # iteration uses q chunk sq=0 (quarter 0).
    gen = attention_gen()

    def advance(n):
        for _ in range(n):
            if next(gen, None) is None:
                break

    # quarter 0: both x DMAs in flight together, then ONLY the (h0, s0) k+q
    # projections (12 matmuls) gate the first exp group — the rest of the
    # quarter follows while ACT chews g0/g1
    FASTSTART = __import__("os").environ.get("BASS_FASTSTART", "1") == "1"
    if FASTSTART:
        xt_k0 = load_xt_quarter("xk", 0)
        xt_q_tiles[0] = load_xt_quarter("xq", 0)
        proj_qk_one(xt_k0, 0, 1, bqbk[:, HPG : 2 * HPG], kT, 0, 0)
        proj_q_group(xt_q_tiles[0], 0, 0, 0)
        advance(1)   # g0: kc 0..2 (kT/qT h0 cols 0..511)
        proj_qk_one(xt_k0, 0, 1, bqbk[:, HPG : 2 * HPG], kT, 0, 1)
        xt_v0 = load_xt_quarter("xv", 0)
        proj_v(xt_v0, 0)
        proj_qk(xt_k0, 0, 1, bqbk[:, HPG : 2 * HPG], kT,
                skip={(0, 0), (0, 1)})
        for h in range(HPG):
            for sqq in range(SQQ):
                if (h, sqq) != (0, 0):
                    proj_q_group(xt_q_tiles[0], 0, h, sqq)
        advance(1)   # g1: kc 3..5 (first PV fires after this exp)
    else:
        xt_k0 = load_xt_quarter("xk", 0)
        proj_qk(xt_k0, 0, 1, bqbk[:, HPG : 2 * HPG], kT)
        xt_q_tiles[0] = load_xt_quarter("xq", 0)
        for h in range(HPG):
            for sqq in range(SQQ):
                proj_q_group(xt_q_tiles[0], 0, h, sqq)
        advance(1)   # g0: kc 0..2 (needs only kT+qT of quarter 0)
        xt_v0 = load_xt_quarter("xv", 0)
        proj_v(xt_v0, 0)
        advance(1)   # g1: kc 3..5 (first PV fires after this exp)
    proj_kv_quarter(1)
    xt_q_tiles[1] = load_xt_quarter("xq", 1)
    if not DEFER_PG:
        for h in range(HPG):
            for sqq in range(SQQ):
                proj_q_group(xt_q_tiles[1], 1, h, sqq)
    advance(3)       # g2..g4: kc 6..14 (quarters 0-1)
    proj_kv_quarter(2)
    xt_q_tiles[2] = load_xt_quarter("xq", 2)
    if not DEFER_PG:
        for h in range(HPG):
            for sqq in range(SQQ):
                proj_q_group(xt_q_tiles[2], 2, h, sqq)
    advance(3)       # g5..g7: kc 15..23 (quarter 2)
    proj_kv_quarter(3)
    xt_q_tiles[3] = load_xt_quarter("xq", 3)
    if not DEFER_PG:
        for h in range(HPG):
            for sqq in range(SQQ):
                proj_q_group(xt_q_tiles[3], 3, h, sqq)
    for _ in gen:
        pass


def _build():
    nc = bacc.Bacc("TRN2", target_bir_lowering=False, debug=False)
    io = {}
    for nm, shape, dt in (
        ("xq", [NDC * S, 128], BF16), ("xk", [NDC * S, 128], BF16),
        ("xv", [NDC * S, 128], BF16),
        ("wq", [D, GD], BF16), ("wk", [D, GD], BF16), ("wv", [D, GD], BF16),
        ("bqbk_pk", [128, 2 * HPG], F32),
        ("bv_r", [1, GD], BF16), ("mask_pk", [128, NKC], F32),
    ):
        io[nm] = nc.dram_tensor(nm, shape, dt, kind="ExternalInput").ap()
    io["out"] = nc.dram_tensor("out", [S, GD], F32, kind="ExternalOutput").ap()

    import os

    dup = int(os.environ.get("BASS_DUP", "1"))
    with tile.TileContext(nc) as tc:
        for _ in range(dup):
            with ExitStack() as ctx:
                _emit(ctx, tc, io)
    nc.compile()
    return nc


_NC = None


def _get_nc():
    global _NC
    if _NC is None:
        _NC = _build()
    return _NC


def make_in_maps(query, key, value, mask, Wq, bq, Wk, bk, Wv, bv):
    bf = lambda a: np.ascontiguousarray(a).astype(BF16_NP)
    bf3 = lambda a: np.ascontiguousarray(
        np.asarray(a).reshape(S, NDC, 128).transpose(1, 0, 2).reshape(NDC * S, 128)
    ).astype(BF16_NP)
    f32 = lambda a: np.ascontiguousarray(np.asarray(a, np.float32))
    in_maps = []
    for c in range(N_CORES):
        b, g = divmod(c, 4)
        cols = slice(g * GD, (g + 1) * GD)
        in_maps.append({
            "xq": bf3(query[b]),
            "xk": bf3(key[b]),
            "xv": bf3(value[b]),
            "wq": bf(Wq[:, cols]),
            "wk": bf(Wk[:, cols]),
            "wv": bf(Wv[:, cols]),
            "bqbk_pk": f32(np.tile(np.concatenate(
                [np.asarray(bq)[cols].reshape(HPG, DK).T,
                 np.asarray(bk)[cols].reshape(HPG, DK).T], axis=1), (2, 1))),
            "bv_r": bf(np.asarray(bv)[cols].reshape(1, GD)),
            "mask_pk": f32(np.asarray(mask)[b].reshape(NKC, 128).T),
        })
    return in_maps


def kernel(query, key, value, mask, Wq, bq, Wk, bk, Wv, bv):
    query = np.asarray(query, np.float32)
    key = np.asarray(key, np.float32)
    value = np.asarray(value, np.float32)
    nc = _get_nc()
    in_maps = make_in_maps(query, key, value, mask, Wq, bq, Wk, bk, Wv, bv)
    res = run_bass_kernel_spmd(nc, in_maps, core_ids=list(range(N_CORES)))
    out = np.empty((B, S, D), np.float32)
    for c in range(N_CORES):
        b, g = divmod(c, 4)
        out[b, :, g * GD : (g + 1) * GD] = res.results[c]["out"]
    return out

